# revision 9
# baseline (speedup 1.0000x reference)
"""Trainium2 Bass kernel for nn_MessageBuildingLayerLSH.

Strategy (8 NeuronCores, data-parallel over batch B=8, one batch element
per core):
  NEFF1 (binning): per 128-node tile, PE-transpose the x_msg tile, f32
    matmul against [codebook, -codebook] (128x200), then DVE max +
    max_index -> argmax in [0,200) (first-index tie-break, matching
    jnp.argmax). Host adds the mask term and does the cheap stable
    argsort -> bins_split permutation.
  NEFF2 (gather + pairwise): dma_gather rows of x_msg/x_node by the
    permutation (int16 indices), mask-multiply, pairwise L2 distances via
    PE matmul + rank-1 PSUM accumulation of the norms, sqrt/exp on ACT,
    mask outer product via rank-1 matmul, write dm and x_features_binned.
"""

import contextlib
import ctypes
import os
import sys
import types
from contextlib import ExitStack

import numpy as np

import concourse.bacc as bacc
import concourse.bass as bass
import concourse.mybir as mybir
import concourse.tile as tile
from concourse.bass_utils import run_bass_kernel_spmd

# ---------------------------------------------------------------- constants
B, N, DM, DN = 8, 25600, 128, 256
BIN, NB, CB = 128, 200, 100     # bin size, n_bins, codebook slice
NT = N // BIN                   # 200 node tiles
GRP = 4                         # bins per group in NEFF2
NG = NB // GRP

F32 = mybir.dt.float32
I16 = mybir.dt.int16
U32 = mybir.dt.uint32
AF = mybir.ActivationFunctionType
ALU = mybir.AluOpType

# ---------------------------------------------------- walrus wait splitter
# This walrus build rejects >1 sem wait per instruction ("Too many sync
# wait commands"). After Tile scheduling, move extra waits onto nop
# carriers inserted just before the instruction on the same engine
# (engines execute block instructions in order, so semantics match).


def _split_multi_waits(nc):
    for fn in nc.m.functions:
        for bb in fn.blocks:
            new = []
            for inst in bb.instructions:
                si = getattr(inst, "sync_info", None)
                if si is not None and si.on_wait and len(si.on_wait) > 1:
                    waits = list(si.on_wait)
                    for i, w in enumerate(waits[:-1]):
                        new.append(mybir.InstNoOp(
                            name=f"{inst.name}-ws{i}",
                            sync_info=mybir.SyncInfo(on_wait=[w],
                                                     on_update=[]),
                            bass_nofuse=True,
                            engine=inst.engine,
                        ))
                    si.on_wait = [waits[-1]]
                new.append(inst)
            bb.instructions[:] = new

# ------------------------------------------------------- NTFF profile shim
# Recreate the missing ``antenv.axon_hooks`` so trace=True can profile.


def _install_ntff_shim():
    if "antenv.axon_hooks" in sys.modules:
        return
    so_path = "/opt/axon/libaxon_pjrt.so"
    hook = None
    try:
        lib = ctypes.CDLL(so_path)
        if hasattr(lib, "axon_start_nrt_profile"):
            lib.axon_start_nrt_profile.argtypes = [
                ctypes.POINTER(ctypes.c_int64),
                ctypes.c_size_t,
            ]
            lib.axon_start_nrt_profile.restype = ctypes.c_int64
            lib.axon_stop_nrt_profile.argtypes = [ctypes.c_char_p]
            lib.axon_stop_nrt_profile.restype = ctypes.c_int64

            @contextlib.contextmanager
            def _hook(output_dir, device_ids):
                import jax

                jax.devices()
                if device_ids:
                    ids = (ctypes.c_int64 * len(device_ids))(*device_ids)
                    rc = lib.axon_start_nrt_profile(ids, len(device_ids))
                else:
                    rc = lib.axon_start_nrt_profile(None, 0)
                if rc != 0:
                    raise RuntimeError(f"axon_start_nrt_profile rc={rc}")
                try:
                    yield
                finally:
                    n = lib.axon_stop_nrt_profile(str(output_dir).encode())
                    if n <= 0:
                        print(f"ntff profile rc={n} -> {output_dir}",
                              file=sys.stderr)

            hook = _hook
    except OSError:
        pass
    mod = types.ModuleType("antenv.axon_hooks")
    mod.get_axon_ntff_profile_hook = lambda: hook
    mod.set_axon_ntff_profile_hook = lambda h: None
    sys.modules["antenv.axon_hooks"] = mod


_install_ntff_shim()


# ------------------------------------------------------------------- NEFF1
def _build_neff1():
    nc = bacc.Bacc("TRN2", target_bir_lowering=False, debug=False, num_devices=B)
    xm = nc.dram_tensor("xm", [N, DM], F32, kind="ExternalInput").ap()
    c2 = nc.dram_tensor("c2", [DM, 2 * CB], F32, kind="ExternalInput").ap()
    ident = nc.dram_tensor("ident", [128, 128], F32, kind="ExternalInput").ap()
    amax = nc.dram_tensor("amax", [NT, BIN, 1], U32, kind="ExternalOutput").ap()

    with tile.TileContext(nc) as tc:
        with ExitStack() as ctx:
            consts = ctx.enter_context(tc.tile_pool(name="consts", bufs=1))
            xpool = ctx.enter_context(tc.tile_pool(name="x", bufs=3))
            ppool = ctx.enter_context(tc.tile_pool(name="ps", bufs=2, space="PSUM"))
            mpool = ctx.enter_context(tc.tile_pool(name="mul", bufs=3))
            opool = ctx.enter_context(tc.tile_pool(name="o", bufs=3))

            c2_sb = consts.tile([DM, 2 * CB], F32)
            nc.sync.dma_start(c2_sb[:], c2[:, :])
            id_sb = consts.tile([128, 128], F32)
            nc.sync.dma_start(id_sb[:], ident[:, :])

            for t in range(NT):
                x_in = xpool.tile([BIN, DM], F32, tag="xin")
                nc.sync.dma_start(x_in[:], xm[t * BIN:(t + 1) * BIN, :])
                xT_ps = ppool.tile([DM, BIN], F32, tag="xT")
                nc.tensor.transpose(xT_ps[:], x_in[:], id_sb[:])
                xT_sb = xpool.tile([DM, BIN], F32, tag="xTsb")
                nc.vector.tensor_copy(xT_sb[:], xT_ps[:])
                mul_ps = ppool.tile([BIN, 2 * CB], F32, tag="mul")
                nc.tensor.matmul(mul_ps[:], xT_sb[:], c2_sb[:], start=True,
                                 stop=True)
                mul_sb = mpool.tile([BIN, 2 * CB], F32, tag="mulsb")
                nc.scalar.copy(mul_sb[:], mul_ps[:])
                mx8 = opool.tile([BIN, 8], F32, tag="mx8")
                nc.vector.max(mx8[:], mul_sb[:])
                ix8 = opool.tile([BIN, 8], U32, tag="ix8")
                nc.vector.max_index(ix8[:], mx8[:], mul_sb[:])
                nc.sync.dma_start(amax[t, :, :], ix8[:, 0:1])
    return nc


# ------------------------------------------------------------------- NEFF2
def _build_neff2():
    nc = bacc.Bacc("TRN2", target_bir_lowering=False, debug=False, num_devices=B)
    xm = nc.dram_tensor("xm", [N, DM], F32, kind="ExternalInput").ap()
    xn = nc.dram_tensor("xn", [N, DN], F32, kind="ExternalInput").ap()
    idx = nc.dram_tensor("idx", [128, N // 16], I16, kind="ExternalInput").ap()
    mskb = nc.dram_tensor("mskb", [NB, BIN], F32, kind="ExternalInput").ap()
    ident = nc.dram_tensor("ident", [128, 128], F32, kind="ExternalInput").ap()
    feat = nc.dram_tensor("feat", [N, DN], F32, kind="ExternalOutput").ap()
    dm = nc.dram_tensor("dm", [NB, BIN, BIN], F32, kind="ExternalOutput").ap()

    NIG = GRP * BIN            # idxs per group (512)
    NIC = NIG // 16            # idx cols per group (32)

    with tile.TileContext(nc) as tc:
        with ExitStack() as ctx:
            consts = ctx.enter_context(tc.tile_pool(name="consts", bufs=1))
            gpool = ctx.enter_context(tc.tile_pool(name="g", bufs=3))
            ppool = ctx.enter_context(tc.tile_pool(name="ps", bufs=2, space="PSUM"))
            wpool = ctx.enter_context(tc.tile_pool(name="w", bufs=3))
            opool = ctx.enter_context(tc.tile_pool(name="o", bufs=3))

            id_sb = consts.tile([128, 128], F32)
            nc.sync.dma_start(id_sb[:], ident[:, :])
            ones_col = consts.tile([128, 1], F32)
            nc.vector.memset(ones_col[:], 1.0)
            ones_row = consts.tile([1, BIN], F32)
            nc.vector.memset(ones_row[:], 1.0)
            idx_sb = consts.tile([128, N // 16], I16)
            nc.sync.dma_start(idx_sb[:], idx[:, :])
            mcol_sb = consts.tile([128, NB], F32)
            nc.sync.dma_start(mcol_sb[:], mskb.rearrange("b n -> n b"))

            feat3 = feat.rearrange("(b n) d -> b n d", n=BIN)
            nig_reg = nc.gpsimd.to_reg(NIG)

            for g in range(NG):
                b0 = g * GRP
                xm_g = gpool.tile([128, GRP, DM], F32, tag="xm_g")
                nc.gpsimd.dma_gather(
                    xm_g[:], xm[:, :], idx_sb[:, g * NIC:(g + 1) * NIC],
                    num_idxs=NIG, num_idxs_reg=nig_reg, elem_size=DM,
                    queue_num=0)
                xn_g = gpool.tile([128, GRP, DN], F32, tag="xn_g")
                nc.gpsimd.dma_gather(
                    xn_g[:], xn[:, :], idx_sb[:, g * NIC:(g + 1) * NIC],
                    num_idxs=NIG, num_idxs_reg=nig_reg, elem_size=DN,
                    queue_num=0)
                nc.sync.dma_start(
                    feat3[b0:b0 + GRP].rearrange("b n d -> n b d"), xn_g[:])

                mrow = wpool.tile([1, NIG], F32, tag="mrow")
                nc.sync.dma_start(
                    mrow[:], mskb[b0:b0 + GRP, :].rearrange("b n -> (b n)")[None, :])

                p_g = ppool.tile([128, GRP, BIN], F32, tag="p")
                m2_g = ppool.tile([128, GRP, BIN], F32, tag="m2")
                n_g = ppool.tile([1, NIG], F32, tag="n")
                xt_sb = wpool.tile([128, GRP, DM], F32, tag="xt")
                sq_sb = wpool.tile([128, GRP, DM], F32, tag="sq")
                nis = wpool.tile([1, NIG], F32, tag="nis")

                for k in range(GRP):
                    bn = b0 + k
                    ks = slice(k * BIN, (k + 1) * BIN)
                    xmm = wpool.tile([128, DM], F32, tag="xmm")
                    nc.vector.tensor_scalar_mul(
                        xmm[:], xm_g[:, k, :], mcol_sb[:, bn:bn + 1])
                    xt_ps = ppool.tile([DM, BIN], F32, tag="xtps")
                    nc.tensor.transpose(xt_ps[:], xmm[:], id_sb[:])
                    nc.vector.tensor_copy(xt_sb[:, k, :], xt_ps[:])
                    nc.scalar.activation(sq_sb[:, k, :], xt_sb[:, k, :],
                                         AF.Square)
                    # n_row[1, BIN] = column sums of sq (= squared norms)
                    nc.tensor.matmul(n_g[:, ks], ones_col[:], sq_sb[:, k, :],
                                     start=True, stop=True)

                # nis = -0.5 * n (batched for the group)
                nc.vector.tensor_scalar_mul(nis[:], n_g[:], -0.5)

                for k in range(GRP):
                    bn = b0 + k
                    ks = slice(k * BIN, (k + 1) * BIN)
                    # P = X.X^T - 0.5*n_j - 0.5*n_i  (so d2 = -2P)
                    nc.tensor.matmul(p_g[:, k, :], xt_sb[:, k, :],
                                     xt_sb[:, k, :], start=True, stop=False)
                    nc.tensor.matmul(p_g[:, k, :], ones_row[:], nis[0:1, ks],
                                     start=False, stop=False)
                    nc.tensor.matmul(p_g[:, k, :], nis[0:1, ks], ones_row[:],
                                     start=False, stop=True)
                    # M2 = m_i * m_j
                    nc.tensor.matmul(m2_g[:, k, :], mrow[0:1, ks],
                                     mrow[0:1, ks], start=True, stop=True)

                u_g = opool.tile([128, GRP, BIN], F32, tag="u")
                nc.vector.tensor_scalar(u_g[:], p_g[:], -2.0, 1e-6,
                                        ALU.mult, ALU.max)
                d_g = opool.tile([128, GRP, BIN], F32, tag="d")
                nc.scalar.activation(d_g[:], u_g[:], AF.Sqrt)
                e_g = opool.tile([128, GRP, BIN], F32, tag="e")
                nc.scalar.activation(e_g[:], d_g[:], AF.Exp, scale=-0.1)
                o_g = opool.tile([128, GRP, BIN], F32, tag="o")
                nc.vector.tensor_mul(o_g[:], e_g[:], m2_g[:])
                nc.sync.dma_start(
                    dm[b0:b0 + GRP].rearrange("b i j -> i b j"), o_g[:])
    return nc


# -------------------------------------------------------------------- host
_NC_CACHE = {}
LAST_EXEC_NS = {}


def _get_nc(name, builder):
    if name not in _NC_CACHE:
        nc = builder()
        nc.finalize()
        _split_multi_waits(nc)
        _NC_CACHE[name] = nc
    return _NC_CACHE[name]


def kernel(x_msg, x_node, msk, codebook):
    x_msg = np.ascontiguousarray(x_msg, dtype=np.float32)
    x_node = np.ascontiguousarray(x_node, dtype=np.float32)
    msk = np.ascontiguousarray(msk, dtype=np.float32)
    codebook = np.ascontiguousarray(codebook, dtype=np.float32)
    trace = os.environ.get("KERNEL_PROFILE") == "1"
    ident = np.eye(128, dtype=np.float32)
    c2 = np.concatenate([codebook[:, :CB], -codebook[:, :CB]], axis=1)
    c2 = np.ascontiguousarray(c2, dtype=np.float32)

    # NEFF1: LSH projection + argmax per node
    nc1 = _get_nc("neff1", _build_neff1)
    in1 = [{"xm": x_msg[b], "c2": c2, "ident": ident} for b in range(B)]
    r1 = run_bass_kernel_spmd(nc1, in1, list(range(B)), trace=trace)
    a = np.stack([r1.results[b]["amax"].reshape(N) for b in range(B)])
    a = a.astype(np.int64)

    # host: mask shift + stable argsort (counting-sort-sized problem)
    bin_idx = a + np.where(msk != 0, 0, NB - 1)
    perm = np.argsort(bin_idx, axis=-1, kind="stable")
    bins_split = perm.reshape(B, NB, BIN).astype(np.int32)
    mskb = np.take_along_axis(msk, perm, axis=1).astype(np.float32)
    idx16 = perm.astype(np.int16).reshape(B, N // 16, 16).transpose(0, 2, 1)
    idx16 = np.ascontiguousarray(np.tile(idx16, (1, 8, 1)))  # (B, 128, N/16)

    # NEFF2: gather + pairwise Gaussian kernel
    nc2 = _get_nc("neff2", _build_neff2)
    in2 = [
        {"xm": x_msg[b], "xn": x_node[b], "idx": idx16[b],
         "mskb": np.ascontiguousarray(mskb[b].reshape(NB, BIN)),
         "ident": ident}
        for b in range(B)
    ]
    r2 = run_bass_kernel_spmd(nc2, in2, list(range(B)), trace=trace)
    feats = np.stack([r2.results[b]["feat"] for b in range(B)])
    feats = feats.reshape(B, NB, BIN, DN)
    dm = np.stack([r2.results[b]["dm"] for b in range(B)])
    dm = dm.reshape(B, NB, BIN, BIN, 1)
    msk_f_binned = mskb.reshape(B, NB, BIN, 1)

    LAST_EXEC_NS.clear()
    LAST_EXEC_NS["neff1"] = r1.exec_time_ns
    LAST_EXEC_NS["neff2"] = r2.exec_time_ns
    return bins_split, feats, dm, msk_f_binned


# revision 11
# speedup vs baseline: 1.1587x; 1.1587x over previous
"""Trainium2 Bass kernel for nn_MessageBuildingLayerLSH.

Strategy (8 NeuronCores, data-parallel over batch B=8, one batch element
per core):
  NEFF1 (binning): per 128-node tile, PE-transpose the x_msg tile, f32
    matmul against [codebook, -codebook] (128x200), then DVE max +
    max_index -> argmax in [0,200) (first-index tie-break, matching
    jnp.argmax). Host adds the mask term and does the cheap stable
    argsort -> bins_split permutation.
  NEFF2 (gather + pairwise): dma_gather rows of x_msg/x_node by the
    permutation (int16 indices), mask-multiply, pairwise L2 distances via
    PE matmul + rank-1 PSUM accumulation of the norms, sqrt/exp on ACT,
    mask outer product via rank-1 matmul, write dm and x_features_binned.
"""

import contextlib
import ctypes
import os
import sys
import types
from contextlib import ExitStack

import numpy as np

import concourse.bacc as bacc
import concourse.bass as bass
import concourse.mybir as mybir
import concourse.tile as tile
from concourse.bass_utils import run_bass_kernel_spmd

# ---------------------------------------------------------------- constants
B, N, DM, DN = 8, 25600, 128, 256
BIN, NB, CB = 128, 200, 100     # bin size, n_bins, codebook slice
NT = N // BIN                   # 200 node tiles
GRP = 4                         # bins per group in NEFF2
NG = NB // GRP

F32 = mybir.dt.float32
I16 = mybir.dt.int16
U32 = mybir.dt.uint32
AF = mybir.ActivationFunctionType
ALU = mybir.AluOpType

# ---------------------------------------------------- walrus wait splitter
# This walrus build rejects >1 sem wait per instruction ("Too many sync
# wait commands"). After Tile scheduling, move extra waits onto nop
# carriers inserted just before the instruction on the same engine
# (engines execute block instructions in order, so semantics match).


def _split_multi_waits(nc):
    for fn in nc.m.functions:
        for bb in fn.blocks:
            new = []
            for inst in bb.instructions:
                si = getattr(inst, "sync_info", None)
                if si is not None and si.on_wait and len(si.on_wait) > 1:
                    waits = list(si.on_wait)
                    for i, w in enumerate(waits[:-1]):
                        new.append(mybir.InstNoOp(
                            name=f"{inst.name}-ws{i}",
                            sync_info=mybir.SyncInfo(on_wait=[w],
                                                     on_update=[]),
                            bass_nofuse=True,
                            engine=inst.engine,
                        ))
                    si.on_wait = [waits[-1]]
                new.append(inst)
            bb.instructions[:] = new

# ------------------------------------------------------- NTFF profile shim
# Recreate the missing ``antenv.axon_hooks`` so trace=True can profile.


def _install_ntff_shim():
    if "antenv.axon_hooks" in sys.modules:
        return
    so_path = "/opt/axon/libaxon_pjrt.so"
    hook = None
    try:
        lib = ctypes.CDLL(so_path)
        if hasattr(lib, "axon_start_nrt_profile"):
            lib.axon_start_nrt_profile.argtypes = [
                ctypes.POINTER(ctypes.c_int64),
                ctypes.c_size_t,
            ]
            lib.axon_start_nrt_profile.restype = ctypes.c_int64
            lib.axon_stop_nrt_profile.argtypes = [ctypes.c_char_p]
            lib.axon_stop_nrt_profile.restype = ctypes.c_int64

            @contextlib.contextmanager
            def _hook(output_dir, device_ids):
                import jax

                jax.devices()
                if device_ids:
                    ids = (ctypes.c_int64 * len(device_ids))(*device_ids)
                    rc = lib.axon_start_nrt_profile(ids, len(device_ids))
                else:
                    rc = lib.axon_start_nrt_profile(None, 0)
                if rc != 0:
                    raise RuntimeError(f"axon_start_nrt_profile rc={rc}")
                try:
                    yield
                finally:
                    n = lib.axon_stop_nrt_profile(str(output_dir).encode())
                    if n <= 0:
                        print(f"ntff profile rc={n} -> {output_dir}",
                              file=sys.stderr)

            hook = _hook
    except OSError:
        pass
    mod = types.ModuleType("antenv.axon_hooks")
    mod.get_axon_ntff_profile_hook = lambda: hook
    mod.set_axon_ntff_profile_hook = lambda h: None
    sys.modules["antenv.axon_hooks"] = mod


_install_ntff_shim()


# ------------------------------------------------------------------- NEFF1
def _build_neff1():
    nc = bacc.Bacc("TRN2", target_bir_lowering=False, debug=False, num_devices=B)
    xmT = nc.dram_tensor("xmT", [DM, N], F32, kind="ExternalInput").ap()
    c2 = nc.dram_tensor("c2", [DM, 2 * CB], F32, kind="ExternalInput").ap()
    amax = nc.dram_tensor("amax", [NT, BIN, 1], U32, kind="ExternalOutput").ap()

    with tile.TileContext(nc) as tc:
        with ExitStack() as ctx:
            consts = ctx.enter_context(tc.tile_pool(name="consts", bufs=1))
            xpool = ctx.enter_context(tc.tile_pool(name="x", bufs=4))
            ppool = ctx.enter_context(tc.tile_pool(name="ps", bufs=3, space="PSUM"))
            mpool = ctx.enter_context(tc.tile_pool(name="mul", bufs=3))
            opool = ctx.enter_context(tc.tile_pool(name="o", bufs=3))

            c2_sb = consts.tile([DM, 2 * CB], F32)
            nc.sync.dma_start(c2_sb[:], c2[:, :])

            for t in range(NT):
                xT_sb = xpool.tile([DM, BIN], F32, tag="xTsb")
                nc.sync.dma_start(xT_sb[:], xmT[:, t * BIN:(t + 1) * BIN])
                mul_ps = ppool.tile([BIN, 2 * CB], F32, tag="mul")
                nc.tensor.matmul(mul_ps[:], xT_sb[:], c2_sb[:], start=True,
                                 stop=True)
                mul_sb = mpool.tile([BIN, 2 * CB], F32, tag="mulsb")
                nc.scalar.copy(mul_sb[:], mul_ps[:])
                mx8 = opool.tile([BIN, 8], F32, tag="mx8")
                nc.vector.max(mx8[:], mul_sb[:])
                ix8 = opool.tile([BIN, 8], U32, tag="ix8")
                nc.vector.max_index(ix8[:], mx8[:], mul_sb[:])
                nc.sync.dma_start(amax[t, :, :], ix8[:, 0:1])
    return nc


# ------------------------------------------------------------------- NEFF2
def _build_neff2():
    nc = bacc.Bacc("TRN2", target_bir_lowering=False, debug=False, num_devices=B)
    xm = nc.dram_tensor("xm", [N, DM], F32, kind="ExternalInput").ap()
    xn = nc.dram_tensor("xn", [N, DN], F32, kind="ExternalInput").ap()
    idx = nc.dram_tensor("idx", [128, N // 16], I16, kind="ExternalInput").ap()
    mskb = nc.dram_tensor("mskb", [NB, BIN], F32, kind="ExternalInput").ap()
    ident = nc.dram_tensor("ident", [128, 128], F32, kind="ExternalInput").ap()
    feat = nc.dram_tensor("feat", [N, DN], F32, kind="ExternalOutput").ap()
    dm = nc.dram_tensor("dm", [NB, BIN, BIN], F32, kind="ExternalOutput").ap()

    NIG = GRP * BIN            # idxs per group (512)
    NIC = NIG // 16            # idx cols per group (32)

    with tile.TileContext(nc) as tc:
        with ExitStack() as ctx:
            consts = ctx.enter_context(tc.tile_pool(name="consts", bufs=1))
            gpool = ctx.enter_context(tc.tile_pool(name="g", bufs=3))
            ppool = ctx.enter_context(tc.tile_pool(name="ps", bufs=2, space="PSUM"))
            wpool = ctx.enter_context(tc.tile_pool(name="w", bufs=3))
            opool = ctx.enter_context(tc.tile_pool(name="o", bufs=3))

            id_sb = consts.tile([128, 128], F32)
            nc.sync.dma_start(id_sb[:], ident[:, :])
            ones_col = consts.tile([128, 1], F32)
            nc.vector.memset(ones_col[:], 1.0)
            ones_row = consts.tile([1, BIN], F32)
            nc.vector.memset(ones_row[:], 1.0)
            idx_sb = consts.tile([128, N // 16], I16)
            nc.sync.dma_start(idx_sb[:], idx[:, :])
            mcol_sb = consts.tile([128, NB], F32)
            nc.sync.dma_start(mcol_sb[:], mskb.rearrange("b n -> n b"))

            feat3 = feat.rearrange("(b n) d -> b n d", n=BIN)
            nig_reg = nc.gpsimd.to_reg(NIG)

            for g in range(NG):
                b0 = g * GRP
                xm_g = gpool.tile([128, GRP, DM], F32, tag="xm_g")
                nc.gpsimd.dma_gather(
                    xm_g[:], xm[:, :], idx_sb[:, g * NIC:(g + 1) * NIC],
                    num_idxs=NIG, num_idxs_reg=nig_reg, elem_size=DM,
                    queue_num=0)
                xn_g = gpool.tile([128, GRP, DN], F32, tag="xn_g")
                nc.gpsimd.dma_gather(
                    xn_g[:], xn[:, :], idx_sb[:, g * NIC:(g + 1) * NIC],
                    num_idxs=NIG, num_idxs_reg=nig_reg, elem_size=DN,
                    queue_num=0)
                nc.sync.dma_start(
                    feat3[b0:b0 + GRP].rearrange("b n d -> n b d"), xn_g[:])

                mrow = wpool.tile([1, NIG], F32, tag="mrow")
                nc.sync.dma_start(
                    mrow[:], mskb[b0:b0 + GRP, :].rearrange("b n -> (b n)")[None, :])

                p_g = ppool.tile([128, GRP, BIN], F32, tag="p")
                m2_g = ppool.tile([128, GRP, BIN], F32, tag="m2")
                n_g = ppool.tile([1, NIG], F32, tag="n")
                xt_sb = wpool.tile([128, GRP, DM], F32, tag="xt")
                sq_sb = wpool.tile([128, GRP, DM], F32, tag="sq")
                nis = wpool.tile([1, NIG], F32, tag="nis")

                for k in range(GRP):
                    bn = b0 + k
                    ks = slice(k * BIN, (k + 1) * BIN)
                    xmm = wpool.tile([128, DM], F32, tag="xmm")
                    nc.vector.tensor_scalar_mul(
                        xmm[:], xm_g[:, k, :], mcol_sb[:, bn:bn + 1])
                    xt_ps = ppool.tile([DM, BIN], F32, tag="xtps")
                    nc.tensor.transpose(xt_ps[:], xmm[:], id_sb[:])
                    nc.vector.tensor_copy(xt_sb[:, k, :], xt_ps[:])
                    nc.scalar.activation(sq_sb[:, k, :], xt_sb[:, k, :],
                                         AF.Square)
                    # n_row[1, BIN] = column sums of sq (= squared norms)
                    nc.tensor.matmul(n_g[:, ks], ones_col[:], sq_sb[:, k, :],
                                     start=True, stop=True)

                # nis = -0.5 * n (batched for the group)
                nc.vector.tensor_scalar_mul(nis[:], n_g[:], -0.5)

                for k in range(GRP):
                    bn = b0 + k
                    ks = slice(k * BIN, (k + 1) * BIN)
                    # P = X.X^T - 0.5*n_j - 0.5*n_i  (so d2 = -2P)
                    nc.tensor.matmul(p_g[:, k, :], xt_sb[:, k, :],
                                     xt_sb[:, k, :], start=True, stop=False)
                    nc.tensor.matmul(p_g[:, k, :], ones_row[:], nis[0:1, ks],
                                     start=False, stop=False)
                    nc.tensor.matmul(p_g[:, k, :], nis[0:1, ks], ones_row[:],
                                     start=False, stop=True)
                    # M2 = m_i * m_j
                    nc.tensor.matmul(m2_g[:, k, :], mrow[0:1, ks],
                                     mrow[0:1, ks], start=True, stop=True)

                u_g = opool.tile([128, GRP, BIN], F32, tag="u")
                nc.vector.tensor_scalar(u_g[:], p_g[:], -2.0, 1e-6,
                                        ALU.mult, ALU.max)
                d_g = opool.tile([128, GRP, BIN], F32, tag="d")
                nc.scalar.activation(d_g[:], u_g[:], AF.Sqrt)
                e_g = opool.tile([128, GRP, BIN], F32, tag="e")
                nc.scalar.activation(e_g[:], d_g[:], AF.Exp, scale=-0.1)
                o_g = opool.tile([128, GRP, BIN], F32, tag="o")
                nc.vector.tensor_mul(o_g[:], e_g[:], m2_g[:])
                nc.sync.dma_start(
                    dm[b0:b0 + GRP].rearrange("b i j -> i b j"), o_g[:])
    return nc


# -------------------------------------------------------------------- host
_NC_CACHE = {}
LAST_EXEC_NS = {}


def _get_nc(name, builder):
    if name not in _NC_CACHE:
        nc = builder()
        nc.finalize()
        _split_multi_waits(nc)
        _NC_CACHE[name] = nc
    return _NC_CACHE[name]


def kernel(x_msg, x_node, msk, codebook):
    x_msg = np.ascontiguousarray(x_msg, dtype=np.float32)
    x_node = np.ascontiguousarray(x_node, dtype=np.float32)
    msk = np.ascontiguousarray(msk, dtype=np.float32)
    codebook = np.ascontiguousarray(codebook, dtype=np.float32)
    trace = os.environ.get("KERNEL_PROFILE") == "1"
    ident = np.eye(128, dtype=np.float32)
    c2 = np.concatenate([codebook[:, :CB], -codebook[:, :CB]], axis=1)
    c2 = np.ascontiguousarray(c2, dtype=np.float32)

    # NEFF1: LSH projection + argmax per node
    nc1 = _get_nc("neff1", _build_neff1)
    xmT = np.ascontiguousarray(x_msg.transpose(0, 2, 1))
    in1 = [{"xmT": xmT[b], "c2": c2} for b in range(B)]
    r1 = run_bass_kernel_spmd(nc1, in1, list(range(B)), trace=trace)
    a = np.stack([r1.results[b]["amax"].reshape(N) for b in range(B)])
    a = a.astype(np.int64)

    # host: mask shift + stable argsort (counting-sort-sized problem)
    bin_idx = a + np.where(msk != 0, 0, NB - 1)
    perm = np.argsort(bin_idx, axis=-1, kind="stable")
    bins_split = perm.reshape(B, NB, BIN).astype(np.int32)
    mskb = np.take_along_axis(msk, perm, axis=1).astype(np.float32)
    idx16 = perm.astype(np.int16).reshape(B, N // 16, 16).transpose(0, 2, 1)
    idx16 = np.ascontiguousarray(np.tile(idx16, (1, 8, 1)))  # (B, 128, N/16)

    # NEFF2: gather + pairwise Gaussian kernel
    nc2 = _get_nc("neff2", _build_neff2)
    in2 = [
        {"xm": x_msg[b], "xn": x_node[b], "idx": idx16[b],
         "mskb": np.ascontiguousarray(mskb[b].reshape(NB, BIN)),
         "ident": ident}
        for b in range(B)
    ]
    r2 = run_bass_kernel_spmd(nc2, in2, list(range(B)), trace=trace)
    feats = np.stack([r2.results[b]["feat"] for b in range(B)])
    feats = feats.reshape(B, NB, BIN, DN)
    dm = np.stack([r2.results[b]["dm"] for b in range(B)])
    dm = dm.reshape(B, NB, BIN, BIN, 1)
    msk_f_binned = mskb.reshape(B, NB, BIN, 1)

    LAST_EXEC_NS.clear()
    LAST_EXEC_NS["neff1"] = r1.exec_time_ns
    LAST_EXEC_NS["neff2"] = r2.exec_time_ns
    LAST_EXEC_NS["r1"] = r1
    LAST_EXEC_NS["r2"] = r2
    return bins_split, feats, dm, msk_f_binned
